# revision 23
# baseline (speedup 1.0000x reference)
"""Trainium2 Bass kernel for nn_Eq1dConv (conv1d(K=3)+bias -> filtered_lrelu).

Math (separable along W; H is untouched because the 2x up/down in H uses a
1-tap filter, so inserted zero rows are dropped again by the ::2 decimate):

  y_b[co,h,m]  = sum_{ci,k} x[ci,h,m+k-1]*w[co,ci,k] + b[co]      (m in [0,512))
  pre_a[m'] = fk1*(y_b[m'-1]+y_b[m'])                  (up-FIR even phase, fk1==fk3)
  pre_b[m'] = fk0*(y_b[m'-1]+y_b[m'+1]) + fk2*y_b[m']  (odd phase, fk0==fk4)
  out[n] = fd0*lr(pre_a[n]) + fd1*lr(pre_b[n]) + fd2*lr(pre_a[n+1]) + fd3*lr(pre_b[n+1])

with lr = leaky-relu(0.2), fk = 4*flip(up_filter), fd = flip(down_filter).

v4: software-pipelined emission. Each granule (2 rowpairs) flows through a
7-stage chain (swdge -> conv -> evict -> s_a/s_b0/u -> prelus -> comb -> og/dma)
spread over 7 emission steps, so every engine's in-order queue only sees ops
whose deps are >= 1 step old. This removes the head-of-line blocking that made
the unskewed version pace at the full chain latency per granule, and keeps the
PE fed (its p-state ramps to 2.4 GHz only while continuously busy).

Engine split:
- PE: 3 conv matmuls + 4 diag(fd) comb matmuls per rowpair (1 PSUM bank each).
- Scalar ACT: single-plane eviction yA[c]=y[c-1]+b (f16), and both lrelus as
  Prelu(in*scale) with scale=fk1/fk0 (alpha=0.2; scale applies pre-activation
  so the negative fk0 is handled exactly).
- DVE: s_a = yA+yA>>1, u = ratio*yB + s_b0 (one STT), og PSUM->SBUF eviction.
- GpSimd: s_b0 = yA + yA>>2 + the SWDGE input DMA (f32->f16 cast in flight).

Sharding: pure data-parallel, batch 8 -> 8 cores, weights replicated.
"""

import numpy as np
from contextlib import ExitStack

import concourse.bass as bass
import concourse.bacc as bacc
import concourse.mybir as mybir
import concourse.tile as tile
from concourse.bass_utils import run_bass_kernel_spmd

B, CIN, COUT, H, W, K = 8, 64, 64, 64, 512, 3
N_CORES = 8
SLOPE = 0.2

F32 = mybir.dt.float32
F16 = mybir.dt.float16
ADD = mybir.AluOpType.add
MULT = mybir.AluOpType.mult
PRELU = mybir.ActivationFunctionType.Prelu
IDENT = mybir.ActivationFunctionType.Identity


def build_program(n_rowpairs=H // 2, rp_per_gran=2):
    """Build the single-core SPMD program. Returns (nc, go)."""
    nc = bacc.Bacc("TRN2", target_bir_lowering=False, debug=False)

    x_d = nc.declare_dram_parameter("x", [CIN, H, W], F32, isOutput=False)
    wb_d = nc.declare_dram_parameter("wb", [K, 128, 128], F16, isOutput=False)
    bcol_d = nc.declare_dram_parameter("bcol", [128, 1], F32, isOutput=False)
    dg_d = nc.declare_dram_parameter("dg", [4, 128, 128], F16, isOutput=False)
    out_d = nc.declare_dram_parameter("out", [COUT, H, W], F32, isOutput=True)

    assert n_rowpairs % rp_per_gran == 0
    n_gran = n_rowpairs // rp_per_gran
    NYB = 4  # yy buffer count
    YW = 520  # per-row width of the shifted-y plane (pads included)
    RP = rp_per_gran

    def go(ratio, fk0, fk1, fk2):
        inv_ratio = fk0 / fk2
        with tile.TileContext(nc) as tc, ExitStack() as ctx:
            cpool = ctx.enter_context(tc.tile_pool(name="consts", bufs=1))
            xpool = ctx.enter_context(tc.tile_pool(name="xg", bufs=4))
            opool = ctx.enter_context(tc.tile_pool(name="og", bufs=4))
            ypool = ctx.enter_context(
                tc.tile_pool(name="ypsum", bufs=2, space=bass.MemorySpace.PSUM)
            )
            fpool = ctx.enter_context(
                tc.tile_pool(name="fpsum", bufs=2, space=bass.MemorySpace.PSUM)
            )
            wkpool = ctx.enter_context(tc.tile_pool(name="work", bufs=4))

            wb_t = []
            for k in range(K):
                t = cpool.tile([128, 128], F16, tag=f"wb{k}")
                nc.sync.dma_start(t[:], wb_d[k])
                wb_t.append(t)
            dg_t = []
            for k in range(4):
                t = cpool.tile([128, 128], F16, tag=f"dg{k}")
                nc.sync.dma_start(t[:], dg_d[k])
                dg_t.append(t)
            bcol = cpool.tile([128, 1], F32, tag="bcol")
            nc.sync.dma_start(bcol[:], bcol_d[:])

            # persistent shifted-y buffers: [128, rp, 2, YW]
            #   plane 0 (A): col j = y_b[j-1]+b  (valid j in [1,513), pads zero)
            #   plane 1 (B): col j = y_b[j]+b    (valid j in [0,512), pads zero)
            yybufs = []
            for i in range(NYB):
                t = cpool.tile([128, RP, 2, YW], F16, tag=f"yy{i}")
                nc.vector.memset(t[:, :, 0, 0:1], 0.0)
                nc.vector.memset(t[:, :, 0, 513:YW], 0.0)
                nc.vector.memset(t[:, :, 1, 512:YW], 0.0)
                # s_b0/s_a read yA up to col WKW+1 = 517 < YW: pads zero
                yybufs.append(t)

            mm = lambda o_, l_, r_, s1, s2: nc.tensor.matmul(
                o_, l_, r_, start=s1, stop=s2
            )

            x_view = x_d.rearrange("c (p hh) w -> (c p) hh w", p=2)
            o_view = out_d.rearrange("c (p hh) w -> (c p) hh w", p=2)

            # cross-step tile handles, keyed by granule index
            xg_t, y_t, sa_t, sb_t, u_t, a2_t, b2_t, f_t, og_t = (
                {}, {}, {}, {}, {}, {}, {}, {}, {}
            )

            def s_swdge(g):
                xg = xpool.tile([128, RP, W], F16, tag="xg")
                nc.gpsimd.dma_start(xg[:], x_view[:, g * RP : (g + 1) * RP, :])
                xg_t[g] = xg

            def s_conv(g):
                xg = xg_t.pop(g)
                y = ypool.tile([128, RP, 512], F32, tag="y", name="y")
                for j in range(RP):  # k=1 (widest range, starts the groups)
                    mm(y[:, j, 0:512], wb_t[1][:], xg[:, j, 0:512], True, False)
                for j in range(RP):  # k=0
                    mm(y[:, j, 1:512], wb_t[0][:], xg[:, j, 0:511], False, False)
                for j in range(RP):  # k=2 (stops the groups)
                    mm(y[:, j, 0:511], wb_t[2][:], xg[:, j, 1:512], False, True)
                y_t[g] = y

            def s_evict(g):
                yy = yybufs[g % NYB]
                # B plane: y+bias at cols [0,512)
                nc.scalar.activation(
                    yy[:, :, 1, 0:512], y_t.pop(g), IDENT,
                    bias=bcol[:, 0:1], scale=1.0,
                )
                # A plane = B shifted right by one col, via SBUF->SBUF DMA
                # (hardware DGE; only costs a kick on the sync queue)
                nc.sync.dma_start(yy[:, :, 0, 1:513], yy[:, :, 1, 0:512])

            WKW = 516  # work-tile row width (4B-aligned row starts)

            def s_sb0(g):
                yA = yybufs[g % NYB][:, :, 0, :]
                s_b0 = wkpool.tile([128, RP, WKW], F16, tag="s_b0")
                # s_b0[m] = y[m-1]+y[m+1] = yA[m]+yA[m+2]; full-width op so
                # the dst stays contiguous (tail cols read zero pads)
                nc.gpsimd.tensor_tensor(
                    s_b0[:], yA[:, :, 0:WKW], yA[:, :, 2 : 2 + WKW], ADD
                )
                sb_t[g] = s_b0

            def s_mid(g):
                yy = yybufs[g % NYB]
                yA = yy[:, :, 0, :]
                yB = yy[:, :, 1, :]
                s_a = wkpool.tile([128, RP, WKW], F16, tag="s_a")
                # s_a[m] = y[m-1]+y[m]  (even offsets -> DVE 16-bit 2x)
                nc.vector.tensor_tensor(
                    s_a[:], yA[:, :, 0:WKW], yB[:, :, 0:WKW], ADD
                )
                u = wkpool.tile([128, RP, WKW], F16, tag="u")
                # u'[m] = (fk0/fk2)*s_b0[m] + y[m]; pre_b = fk2*u'
                # (contiguous in0 = s_b0; the strided yB rides as in1)
                nc.vector.scalar_tensor_tensor(
                    u[:], sb_t.pop(g)[:], float(inv_ratio),
                    yB[:, :, 0:WKW], MULT, ADD,
                )
                sa_t[g], u_t[g] = s_a, u

            def s_act(g):
                s_a = sa_t.pop(g)
                u = u_t.pop(g)
                a2 = wkpool.tile([128, RP, WKW], F16, tag="a2")
                nc.scalar.activation(
                    a2[:], s_a[:], PRELU, bias=0.0, scale=float(fk1), alpha=SLOPE
                )
                b2 = wkpool.tile([128, RP, WKW], F16, tag="b2")
                nc.scalar.activation(
                    b2[:], u[:], PRELU, bias=0.0, scale=float(fk2), alpha=SLOPE
                )
                a2_t[g], b2_t[g] = a2, b2

            def s_comb(g):
                a2 = a2_t.pop(g)
                b2 = b2_t.pop(g)
                f = fpool.tile([128, RP, 512], F32, tag="f", name="f")
                for j in range(RP):
                    mm(f[:, j, :], dg_t[0][:], a2[:, j, 0:512], True, False)
                for j in range(RP):
                    mm(f[:, j, :], dg_t[1][:], b2[:, j, 0:512], False, False)
                for j in range(RP):
                    mm(f[:, j, :], dg_t[2][:], a2[:, j, 1:513], False, False)
                for j in range(RP):
                    mm(f[:, j, :], dg_t[3][:], b2[:, j, 1:513], False, True)
                f_t[g] = f

            def s_og(g):
                og = opool.tile([128, RP, W], F32, tag="og")
                nc.vector.tensor_scalar(og[:], f_t.pop(g), 1.0, None, MULT)
                og_t[g] = og

            def s_dma(g):
                nc.sync.dma_start(
                    o_view[:, g * RP : (g + 1) * RP, :], og_t.pop(g)
                )

            def live(g):
                return 0 <= g < n_gran

            # software-pipelined emission: per engine, older-granule ops whose
            # deps are already settled come first so nothing head-of-line
            # blocks behind a same-step producer on another engine.
            for t in range(n_gran + 7):
                if live(t - 6):
                    s_og(t - 6)       # DVE: deps (comb t-6) one step old
                if live(t - 3):
                    s_sb0(t - 3)      # gpsimd: dep (evict t-3) one step old
                if live(t):
                    s_swdge(t)        # gpsimd queue kick
                if live(t - 5):
                    s_comb(t - 5)     # PE: drain old granule first
                if live(t - 1):
                    s_conv(t - 1)     # PE
                if live(t - 2):
                    s_evict(t - 2)    # scalar first: gates next step's s_mid
                if live(t - 3):
                    s_mid(t - 3)      # DVE: s_a, then u (waits gpsimd s_b0)
                if live(t - 4):
                    s_act(t - 4)      # scalar: a2 + b2 Prelus
                if live(t - 6):
                    s_dma(t - 6)      # sync queue

    return nc, go


def derive_consts(conv_w, bias, up_filter, down_filter):
    f = np.asarray(up_filter, dtype=np.float64).reshape(-1)
    d = np.asarray(down_filter, dtype=np.float64).reshape(-1)
    fk = (f * 4.0)[::-1]
    fd = d[::-1]
    assert abs(fk[1] - fk[3]) < 1e-6 * max(1.0, abs(fk[1])), "up filter not symmetric"
    assert abs(fk[0] - fk[4]) < 1e-6 * max(1.0, abs(fk[0])), "up filter not symmetric"
    fk0, fk1, fk2 = float(fk[0]), float(fk[1]), float(fk[2])
    assert fk0 != 0.0
    ratio = fk2 / fk0

    # partition index q = 2*ci + g (g = h-half); output partition 2*co + g
    cw = np.asarray(conv_w, dtype=np.float32)  # [co, ci, 1, K]
    wb = np.zeros((K, 128, 128), dtype=np.float16)
    for k in range(K):
        wk = cw[:, :, 0, k].T.astype(np.float16)  # [ci, co]
        wb[k, 0::2, 0::2] = wk
        wb[k, 1::2, 1::2] = wk

    bcol = np.repeat(np.asarray(bias, dtype=np.float32), 2).reshape(128, 1)

    # comb taps are plain fd (fk scales are applied inside the Prelus)
    eye = np.eye(128, dtype=np.float32)
    dg = np.stack(
        [
            np.float32(fd[0]) * eye,
            np.float32(fd[1]) * eye,
            np.float32(fd[2]) * eye,
            np.float32(fd[3]) * eye,
        ]
    ).astype(np.float16)

    return {
        "wb": wb,
        "bcol": bcol,
        "dg": dg,
        "ratio": ratio,
        "fk0": fk0,
        "fk1": fk1,
        "fk2": fk2,
    }


_CACHE = {}


def _get_compiled(consts_key, ratio, fk0, fk1, fk2):
    if consts_key in _CACHE:
        return _CACHE[consts_key]
    nc, go = build_program()
    go(ratio, fk0, fk1, fk2)
    nc.compile()
    _CACHE[consts_key] = nc
    return nc


def run(x, conv_w, bias, up_filter, down_filter, trace=False, **trace_kw):
    x = np.asarray(x, dtype=np.float32)
    c = derive_consts(conv_w, bias, up_filter, down_filter)

    key = (float(c["ratio"]), float(c["fk0"]), float(c["fk1"]), float(c["fk2"]))
    nc = _get_compiled(key, c["ratio"], c["fk0"], c["fk1"], c["fk2"])

    in_maps = []
    for i in range(N_CORES):
        in_maps.append(
            {
                "x": np.ascontiguousarray(x[i]),
                "wb": c["wb"],
                "bcol": c["bcol"],
                "dg": c["dg"],
            }
        )
    res = run_bass_kernel_spmd(
        nc, in_maps, list(range(N_CORES)), trace=trace, **trace_kw
    )
    out = np.stack([res.results[i]["out"] for i in range(N_CORES)], axis=0)
    return out.astype(np.float32), res


def kernel(x, conv_w, bias, up_filter, down_filter):
    out, _ = run(x, conv_w, bias, up_filter, down_filter)
    return out


# revision 24
# speedup vs baseline: 1.0564x; 1.0564x over previous
"""Trainium2 Bass kernel for nn_Eq1dConv (conv1d(K=3)+bias -> filtered_lrelu).

Math (separable along W; H is untouched because the 2x up/down in H uses a
1-tap filter, so inserted zero rows are dropped again by the ::2 decimate):

  y_b[co,h,m]  = sum_{ci,k} x[ci,h,m+k-1]*w[co,ci,k] + b[co]      (m in [0,512))
  pre_a[m'] = fk1*(y_b[m'-1]+y_b[m'])                  (up-FIR even phase, fk1==fk3)
  pre_b[m'] = fk0*(y_b[m'-1]+y_b[m'+1]) + fk2*y_b[m']  (odd phase, fk0==fk4)
  out[n] = fd0*lr(pre_a[n]) + fd1*lr(pre_b[n]) + fd2*lr(pre_a[n+1]) + fd3*lr(pre_b[n+1])

with lr = leaky-relu(0.2), fk = 4*flip(up_filter), fd = flip(down_filter).

v4: software-pipelined emission. Each granule (2 rowpairs) flows through a
7-stage chain (swdge -> conv -> evict -> s_a/s_b0/u -> prelus -> comb -> og/dma)
spread over 7 emission steps, so every engine's in-order queue only sees ops
whose deps are >= 1 step old. This removes the head-of-line blocking that made
the unskewed version pace at the full chain latency per granule, and keeps the
PE fed (its p-state ramps to 2.4 GHz only while continuously busy).

Engine split:
- PE: 3 conv matmuls + 4 diag(fd) comb matmuls per rowpair (1 PSUM bank each).
- Scalar ACT: single-plane eviction yA[c]=y[c-1]+b (f16), and both lrelus as
  Prelu(in*scale) with scale=fk1/fk0 (alpha=0.2; scale applies pre-activation
  so the negative fk0 is handled exactly).
- DVE: s_a = yA+yA>>1, u = ratio*yB + s_b0 (one STT), og PSUM->SBUF eviction.
- GpSimd: s_b0 = yA + yA>>2 + the SWDGE input DMA (f32->f16 cast in flight).

Sharding: pure data-parallel, batch 8 -> 8 cores, weights replicated.
"""

import numpy as np
from contextlib import ExitStack

import concourse.bass as bass
import concourse.bacc as bacc
import concourse.mybir as mybir
import concourse.tile as tile
from concourse.bass_utils import run_bass_kernel_spmd

B, CIN, COUT, H, W, K = 8, 64, 64, 64, 512, 3
N_CORES = 8
SLOPE = 0.2

F32 = mybir.dt.float32
F16 = mybir.dt.float16
ADD = mybir.AluOpType.add
MULT = mybir.AluOpType.mult
PRELU = mybir.ActivationFunctionType.Prelu
IDENT = mybir.ActivationFunctionType.Identity


def build_program(n_rowpairs=H // 2, rp_per_gran=2):
    """Build the single-core SPMD program. Returns (nc, go)."""
    nc = bacc.Bacc("TRN2", target_bir_lowering=False, debug=False)

    x_d = nc.declare_dram_parameter("x", [CIN, H, W], F32, isOutput=False)
    wb_d = nc.declare_dram_parameter("wb", [K, 128, 128], F16, isOutput=False)
    bcol_d = nc.declare_dram_parameter("bcol", [128, 1], F32, isOutput=False)
    dg_d = nc.declare_dram_parameter("dg", [4, 128, 128], F16, isOutput=False)
    out_d = nc.declare_dram_parameter("out", [COUT, H, W], F32, isOutput=True)

    assert n_rowpairs % rp_per_gran == 0
    n_gran = n_rowpairs // rp_per_gran
    NYB = 4  # yy buffer count
    YW = 520  # per-row width of the shifted-y plane (pads included)
    RP = rp_per_gran

    def go(ratio, fk0, fk1, fk2):
        inv_ratio = fk0 / fk2
        with tile.TileContext(nc) as tc, ExitStack() as ctx:
            cpool = ctx.enter_context(tc.tile_pool(name="consts", bufs=1))
            xpool = ctx.enter_context(tc.tile_pool(name="xg", bufs=4))
            opool = ctx.enter_context(tc.tile_pool(name="og", bufs=4))
            ypool = ctx.enter_context(
                tc.tile_pool(name="ypsum", bufs=2, space=bass.MemorySpace.PSUM)
            )
            fpool = ctx.enter_context(
                tc.tile_pool(name="fpsum", bufs=2, space=bass.MemorySpace.PSUM)
            )
            wkpool = ctx.enter_context(tc.tile_pool(name="work", bufs=4))

            wb_t = []
            for k in range(K):
                t = cpool.tile([128, 128], F16, tag=f"wb{k}")
                nc.sync.dma_start(t[:], wb_d[k])
                wb_t.append(t)
            dg_t = []
            for k in range(4):
                t = cpool.tile([128, 128], F16, tag=f"dg{k}")
                nc.sync.dma_start(t[:], dg_d[k])
                dg_t.append(t)
            bcol = cpool.tile([128, 1], F32, tag="bcol")
            nc.sync.dma_start(bcol[:], bcol_d[:])

            # persistent shifted-y planes: [128, rp, YW]
            #   Q[c] = y_b[c-2]+b  (valid c in [2,514); pads [0,2) and
            #   [514,YW) stay zero)
            yybufs = []
            for i in range(NYB):
                t = cpool.tile([128, RP, YW], F16, tag=f"yy{i}")
                nc.vector.memset(t[:, :, 0:2], 0.0)
                nc.vector.memset(t[:, :, 514:YW], 0.0)
                yybufs.append(t)

            mm = lambda o_, l_, r_, s1, s2: nc.tensor.matmul(
                o_, l_, r_, start=s1, stop=s2
            )

            x_view = x_d.rearrange("c (p hh) w -> (c p) hh w", p=2)
            o_view = out_d.rearrange("c (p hh) w -> (c p) hh w", p=2)

            # cross-step tile handles, keyed by granule index
            xg_t, y_t, sa_t, sb_t, u_t, a2_t, b2_t, f_t, og_t = (
                {}, {}, {}, {}, {}, {}, {}, {}, {}
            )

            def s_swdge(g):
                xg = xpool.tile([128, RP, W], F16, tag="xg")
                nc.gpsimd.dma_start(xg[:], x_view[:, g * RP : (g + 1) * RP, :])
                xg_t[g] = xg

            def s_conv(g):
                xg = xg_t.pop(g)
                y = ypool.tile([128, RP, 512], F32, tag="y", name="y")
                for j in range(RP):  # k=1 (widest range, starts the groups)
                    mm(y[:, j, 0:512], wb_t[1][:], xg[:, j, 0:512], True, False)
                for j in range(RP):  # k=0
                    mm(y[:, j, 1:512], wb_t[0][:], xg[:, j, 0:511], False, False)
                for j in range(RP):  # k=2 (stops the groups)
                    mm(y[:, j, 0:511], wb_t[2][:], xg[:, j, 1:512], False, True)
                y_t[g] = y

            def s_evict(g):
                yy = yybufs[g % NYB]
                # Q[c] = y[c-2]+b on cols [2,514)
                nc.scalar.activation(
                    yy[:, :, 2:514], y_t.pop(g), IDENT,
                    bias=bcol[:, 0:1], scale=1.0,
                )

            def s_sb0(g):
                Q = yybufs[g % NYB]
                s_b0 = wkpool.tile([128, RP, 513], F16, tag="s_b0")
                # s_b0[m] = y[m-1]+y[m+1] = Q[m+1]+Q[m+3]
                nc.gpsimd.tensor_tensor(
                    s_b0[:], Q[:, :, 1:514], Q[:, :, 3:516], ADD
                )
                sb_t[g] = s_b0

            def s_mid(g):
                Q = yybufs[g % NYB]
                s_a = wkpool.tile([128, RP, 513], F16, tag="s_a")
                # s_a[m] = y[m-1]+y[m] = Q[m+1]+Q[m+2]
                nc.vector.tensor_tensor(
                    s_a[:], Q[:, :, 1:514], Q[:, :, 2:515], ADD
                )
                u = wkpool.tile([128, RP, 513], F16, tag="u")
                # u'[m] = (fk0/fk2)*s_b0[m] + y[m]; pre_b = fk2*u'
                # (contiguous in0 = s_b0; y[m] = Q[m+2], even offset)
                nc.vector.scalar_tensor_tensor(
                    u[:], sb_t.pop(g)[:], float(inv_ratio),
                    Q[:, :, 2:515], MULT, ADD,
                )
                sa_t[g], u_t[g] = s_a, u

            def s_act(g):
                s_a = sa_t.pop(g)
                u = u_t.pop(g)
                a2 = wkpool.tile([128, RP, 513], F16, tag="a2")
                nc.scalar.activation(
                    a2[:], s_a[:], PRELU, bias=0.0, scale=float(fk1), alpha=SLOPE
                )
                b2 = wkpool.tile([128, RP, 513], F16, tag="b2")
                nc.scalar.activation(
                    b2[:], u[:], PRELU, bias=0.0, scale=float(fk2), alpha=SLOPE
                )
                a2_t[g], b2_t[g] = a2, b2

            def s_comb(g):
                a2 = a2_t.pop(g)
                b2 = b2_t.pop(g)
                f = fpool.tile([128, RP, 512], F32, tag="f", name="f")
                for j in range(RP):
                    mm(f[:, j, :], dg_t[0][:], a2[:, j, 0:512], True, False)
                for j in range(RP):
                    mm(f[:, j, :], dg_t[1][:], b2[:, j, 0:512], False, False)
                for j in range(RP):
                    mm(f[:, j, :], dg_t[2][:], a2[:, j, 1:513], False, False)
                for j in range(RP):
                    mm(f[:, j, :], dg_t[3][:], b2[:, j, 1:513], False, True)
                f_t[g] = f

            def s_og(g):
                og = opool.tile([128, RP, W], F32, tag="og")
                nc.vector.tensor_scalar(og[:], f_t.pop(g), 1.0, None, MULT)
                og_t[g] = og

            def s_dma(g):
                nc.sync.dma_start(
                    o_view[:, g * RP : (g + 1) * RP, :], og_t.pop(g)
                )

            def live(g):
                return 0 <= g < n_gran

            # software-pipelined emission: per engine, older-granule ops whose
            # deps are already settled come first so nothing head-of-line
            # blocks behind a same-step producer on another engine.
            for t in range(n_gran + 7):
                if live(t - 6):
                    s_og(t - 6)       # DVE: deps (comb t-6) one step old
                if live(t - 3):
                    s_sb0(t - 3)      # gpsimd: dep (evict t-3) one step old
                if live(t):
                    s_swdge(t)        # gpsimd queue kick
                if live(t - 5):
                    s_comb(t - 5)     # PE: drain old granule first
                if live(t - 1):
                    s_conv(t - 1)     # PE
                if live(t - 2):
                    s_evict(t - 2)    # scalar first: gates next step's s_mid
                if live(t - 3):
                    s_mid(t - 3)      # DVE: s_a, then u (waits gpsimd s_b0)
                if live(t - 4):
                    s_act(t - 4)      # scalar: a2 + b2 Prelus
                if live(t - 6):
                    s_dma(t - 6)      # sync queue

    return nc, go


def derive_consts(conv_w, bias, up_filter, down_filter):
    f = np.asarray(up_filter, dtype=np.float64).reshape(-1)
    d = np.asarray(down_filter, dtype=np.float64).reshape(-1)
    fk = (f * 4.0)[::-1]
    fd = d[::-1]
    assert abs(fk[1] - fk[3]) < 1e-6 * max(1.0, abs(fk[1])), "up filter not symmetric"
    assert abs(fk[0] - fk[4]) < 1e-6 * max(1.0, abs(fk[0])), "up filter not symmetric"
    fk0, fk1, fk2 = float(fk[0]), float(fk[1]), float(fk[2])
    assert fk0 != 0.0
    ratio = fk2 / fk0

    # partition index q = 2*ci + g (g = h-half); output partition 2*co + g
    cw = np.asarray(conv_w, dtype=np.float32)  # [co, ci, 1, K]
    wb = np.zeros((K, 128, 128), dtype=np.float16)
    for k in range(K):
        wk = cw[:, :, 0, k].T.astype(np.float16)  # [ci, co]
        wb[k, 0::2, 0::2] = wk
        wb[k, 1::2, 1::2] = wk

    bcol = np.repeat(np.asarray(bias, dtype=np.float32), 2).reshape(128, 1)

    # comb taps are plain fd (fk scales are applied inside the Prelus)
    eye = np.eye(128, dtype=np.float32)
    dg = np.stack(
        [
            np.float32(fd[0]) * eye,
            np.float32(fd[1]) * eye,
            np.float32(fd[2]) * eye,
            np.float32(fd[3]) * eye,
        ]
    ).astype(np.float16)

    return {
        "wb": wb,
        "bcol": bcol,
        "dg": dg,
        "ratio": ratio,
        "fk0": fk0,
        "fk1": fk1,
        "fk2": fk2,
    }


_CACHE = {}


def _get_compiled(consts_key, ratio, fk0, fk1, fk2):
    if consts_key in _CACHE:
        return _CACHE[consts_key]
    nc, go = build_program()
    go(ratio, fk0, fk1, fk2)
    nc.compile()
    _CACHE[consts_key] = nc
    return nc


def run(x, conv_w, bias, up_filter, down_filter, trace=False, **trace_kw):
    x = np.asarray(x, dtype=np.float32)
    c = derive_consts(conv_w, bias, up_filter, down_filter)

    key = (float(c["ratio"]), float(c["fk0"]), float(c["fk1"]), float(c["fk2"]))
    nc = _get_compiled(key, c["ratio"], c["fk0"], c["fk1"], c["fk2"])

    in_maps = []
    for i in range(N_CORES):
        in_maps.append(
            {
                "x": np.ascontiguousarray(x[i]),
                "wb": c["wb"],
                "bcol": c["bcol"],
                "dg": c["dg"],
            }
        )
    res = run_bass_kernel_spmd(
        nc, in_maps, list(range(N_CORES)), trace=trace, **trace_kw
    )
    out = np.stack([res.results[i]["out"] for i in range(N_CORES)], axis=0)
    return out.astype(np.float32), res


def kernel(x, conv_w, bias, up_filter, down_filter):
    out, _ = run(x, conv_w, bias, up_filter, down_filter)
    return out


# revision 25
# speedup vs baseline: 1.1329x; 1.0724x over previous
"""Trainium2 Bass kernel for nn_Eq1dConv (conv1d(K=3)+bias -> filtered_lrelu).

Math (separable along W; H is untouched because the 2x up/down in H uses a
1-tap filter, so inserted zero rows are dropped again by the ::2 decimate):

  y_b[co,h,m]  = sum_{ci,k} x[ci,h,m+k-1]*w[co,ci,k] + b[co]      (m in [0,512))
  pre_a[m'] = fk1*(y_b[m'-1]+y_b[m'])                  (up-FIR even phase, fk1==fk3)
  pre_b[m'] = fk0*(y_b[m'-1]+y_b[m'+1]) + fk2*y_b[m']  (odd phase, fk0==fk4)
  out[n] = fd0*lr(pre_a[n]) + fd1*lr(pre_b[n]) + fd2*lr(pre_a[n+1]) + fd3*lr(pre_b[n+1])

with lr = leaky-relu(0.2), fk = 4*flip(up_filter), fd = flip(down_filter).

v4: software-pipelined emission. Each granule (2 rowpairs) flows through a
7-stage chain (swdge -> conv -> evict -> s_a/s_b0/u -> prelus -> comb -> og/dma)
spread over 7 emission steps, so every engine's in-order queue only sees ops
whose deps are >= 1 step old. This removes the head-of-line blocking that made
the unskewed version pace at the full chain latency per granule, and keeps the
PE fed (its p-state ramps to 2.4 GHz only while continuously busy).

Engine split:
- PE: 3 conv matmuls + 4 diag(fd) comb matmuls per rowpair (1 PSUM bank each).
- Scalar ACT: single-plane eviction yA[c]=y[c-1]+b (f16), and both lrelus as
  Prelu(in*scale) with scale=fk1/fk0 (alpha=0.2; scale applies pre-activation
  so the negative fk0 is handled exactly).
- DVE: s_a = yA+yA>>1, u = ratio*yB + s_b0 (one STT), og PSUM->SBUF eviction.
- GpSimd: s_b0 = yA + yA>>2 + the SWDGE input DMA (f32->f16 cast in flight).

Sharding: pure data-parallel, batch 8 -> 8 cores, weights replicated.
"""

import numpy as np
from contextlib import ExitStack

import concourse.bass as bass
import concourse.bacc as bacc
import concourse.mybir as mybir
import concourse.tile as tile
from concourse.bass_utils import run_bass_kernel_spmd

B, CIN, COUT, H, W, K = 8, 64, 64, 64, 512, 3
N_CORES = 8
SLOPE = 0.2

F32 = mybir.dt.float32
F16 = mybir.dt.float16
ADD = mybir.AluOpType.add
MULT = mybir.AluOpType.mult
PRELU = mybir.ActivationFunctionType.Prelu
IDENT = mybir.ActivationFunctionType.Identity


def build_program(n_rowpairs=H // 2, rp_per_gran=2):
    """Build the single-core SPMD program. Returns (nc, go)."""
    nc = bacc.Bacc("TRN2", target_bir_lowering=False, debug=False)

    x_d = nc.declare_dram_parameter("x", [CIN, H, W], F32, isOutput=False)
    wb_d = nc.declare_dram_parameter("wb", [K, 128, 128], F16, isOutput=False)
    bcol_d = nc.declare_dram_parameter("bcol", [128, 1], F32, isOutput=False)
    dg_d = nc.declare_dram_parameter("dg", [4, 128, 128], F16, isOutput=False)
    out_d = nc.declare_dram_parameter("out", [COUT, H, W], F32, isOutput=True)

    assert n_rowpairs % rp_per_gran == 0
    n_gran = n_rowpairs // rp_per_gran
    NYB = 4  # yy buffer count
    YW = 520  # per-row width of the shifted-y plane (pads included)
    RP = rp_per_gran

    def go(ratio, fk0, fk1, fk2):
        inv_ratio = fk0 / fk2
        with tile.TileContext(nc) as tc, ExitStack() as ctx:
            cpool = ctx.enter_context(tc.tile_pool(name="consts", bufs=1))
            xpool = ctx.enter_context(tc.tile_pool(name="xg", bufs=4))
            opool = ctx.enter_context(tc.tile_pool(name="og", bufs=4))
            ypool = ctx.enter_context(
                tc.tile_pool(name="ypsum", bufs=2, space=bass.MemorySpace.PSUM)
            )
            fpool = ctx.enter_context(
                tc.tile_pool(name="fpsum", bufs=2, space=bass.MemorySpace.PSUM)
            )
            wkpool = ctx.enter_context(tc.tile_pool(name="work", bufs=4))

            wb_t = []
            for k in range(K):
                t = cpool.tile([128, 128], F16, tag=f"wb{k}")
                nc.sync.dma_start(t[:], wb_d[k])
                wb_t.append(t)
            dg_t = []
            for k in range(4):
                t = cpool.tile([128, 128], F16, tag=f"dg{k}")
                nc.sync.dma_start(t[:], dg_d[k])
                dg_t.append(t)
            bcol = cpool.tile([128, 1], F32, tag="bcol")
            nc.sync.dma_start(bcol[:], bcol_d[:])

            # persistent shifted-y buffers: [128, rp, 2, YW]
            #   plane 0 (A): col j = y_b[j-1]+b  (valid j in [1,513), pads zero)
            #   plane 1 (B): col j = y_b[j]+b    (valid j in [0,512), pads zero)
            yybufs = []
            for i in range(NYB):
                t = cpool.tile([128, RP, 2, YW], F16, tag=f"yy{i}")
                nc.vector.memset(t[:, :, 0, 0:1], 0.0)
                nc.vector.memset(t[:, :, 0, 513:YW], 0.0)
                nc.vector.memset(t[:, :, 1, 512:YW], 0.0)
                yybufs.append(t)

            mm = lambda o_, l_, r_, s1, s2: nc.tensor.matmul(
                o_, l_, r_, start=s1, stop=s2
            )

            x_view = x_d.rearrange("c (p hh) w -> (c p) hh w", p=2)
            o_view = out_d.rearrange("c (p hh) w -> (c p) hh w", p=2)

            # cross-step tile handles, keyed by granule index
            xg_t, y_t, sa_t, sb_t, u_t, a2_t, b2_t, f_t, og_t = (
                {}, {}, {}, {}, {}, {}, {}, {}, {}
            )

            def s_swdge(g):
                xg = xpool.tile([128, RP, W], F16, tag="xg")
                nc.gpsimd.dma_start(xg[:], x_view[:, g * RP : (g + 1) * RP, :])
                xg_t[g] = xg

            def s_conv(g):
                xg = xg_t.pop(g)
                y = ypool.tile([128, RP, 512], F32, tag="y", name="y")
                for j in range(RP):  # k=1 (widest range, starts the groups)
                    mm(y[:, j, 0:512], wb_t[1][:], xg[:, j, 0:512], True, False)
                for j in range(RP):  # k=0
                    mm(y[:, j, 1:512], wb_t[0][:], xg[:, j, 0:511], False, False)
                for j in range(RP):  # k=2 (stops the groups)
                    mm(y[:, j, 0:511], wb_t[2][:], xg[:, j, 1:512], False, True)
                y_t[g] = y

            def s_evict(g):
                yy = yybufs[g % NYB]
                # ONE ACT op: y+bias -> both shifted fp16 planes of yy
                flat3 = yy.rearrange("p a b c -> p a (b c)")  # [128, rp, 1040]
                dual = flat3[:, :, 1 : 1 + 2 * 519].rearrange(
                    "p a (r c) -> p a r c", c=519
                )[:, :, :, 0:512]
                src = y_t.pop(g).unsqueeze(2).broadcast_to([128, RP, 2, 512])
                nc.scalar.activation(dual, src, IDENT, bias=bcol[:, 0:1], scale=1.0)

            def s_sb0(g):
                yA = yybufs[g % NYB][:, :, 0, :]
                s_b0 = wkpool.tile([128, RP, 513], F16, tag="s_b0")
                # s_b0[m] = y[m-1]+y[m+1] = yA[m]+yA[m+2]
                nc.gpsimd.tensor_tensor(
                    s_b0[:], yA[:, :, 0:513], yA[:, :, 2:515], ADD
                )
                sb_t[g] = s_b0

            def s_mid(g):
                yy = yybufs[g % NYB]
                yA = yy[:, :, 0, :]
                yB = yy[:, :, 1, :]
                s_a = wkpool.tile([128, RP, 513], F16, tag="s_a")
                # s_a[m] = y[m-1]+y[m]  (even offsets -> DVE 16-bit 2x)
                nc.vector.tensor_tensor(
                    s_a[:], yA[:, :, 0:513], yB[:, :, 0:513], ADD
                )
                u = wkpool.tile([128, RP, 513], F16, tag="u")
                # u'[m] = (fk0/fk2)*s_b0[m] + y[m]; pre_b = fk2*u'
                # (contiguous in0 = s_b0; the strided yB rides as in1)
                nc.vector.scalar_tensor_tensor(
                    u[:], sb_t.pop(g), float(inv_ratio), yB[:, :, 0:513], MULT, ADD
                )
                sa_t[g], u_t[g] = s_a, u

            def s_act(g):
                s_a = sa_t.pop(g)
                u = u_t.pop(g)
                a2 = wkpool.tile([128, RP, 513], F16, tag="a2")
                nc.scalar.activation(
                    a2[:], s_a[:], PRELU, bias=0.0, scale=float(fk1), alpha=SLOPE
                )
                b2 = wkpool.tile([128, RP, 513], F16, tag="b2")
                nc.scalar.activation(
                    b2[:], u[:], PRELU, bias=0.0, scale=float(fk2), alpha=SLOPE
                )
                a2_t[g], b2_t[g] = a2, b2

            def s_comb(g):
                a2 = a2_t.pop(g)
                b2 = b2_t.pop(g)
                f = fpool.tile([128, RP, 512], F32, tag="f", name="f")
                for j in range(RP):
                    mm(f[:, j, :], dg_t[0][:], a2[:, j, 0:512], True, False)
                for j in range(RP):
                    mm(f[:, j, :], dg_t[1][:], b2[:, j, 0:512], False, False)
                for j in range(RP):
                    mm(f[:, j, :], dg_t[2][:], a2[:, j, 1:513], False, False)
                for j in range(RP):
                    mm(f[:, j, :], dg_t[3][:], b2[:, j, 1:513], False, True)
                f_t[g] = f

            def s_og(g):
                og = opool.tile([128, RP, W], F32, tag="og")
                nc.vector.tensor_scalar(og[:], f_t.pop(g), 1.0, None, MULT)
                og_t[g] = og

            def s_dma(g):
                nc.sync.dma_start(
                    o_view[:, g * RP : (g + 1) * RP, :], og_t.pop(g)
                )

            def live(g):
                return 0 <= g < n_gran

            # software-pipelined emission: per engine, older-granule ops whose
            # deps are already settled come first so nothing head-of-line
            # blocks behind a same-step producer on another engine.
            for t in range(n_gran + 7):
                if live(t - 6):
                    s_og(t - 6)       # DVE: deps (comb t-6) one step old
                if live(t - 3):
                    s_sb0(t - 3)      # gpsimd: dep (evict t-3) one step old
                if live(t):
                    s_swdge(t)        # gpsimd queue kick
                if live(t - 5):
                    s_comb(t - 5)     # PE: drain old granule first
                if live(t - 1):
                    s_conv(t - 1)     # PE
                if live(t - 2):
                    s_evict(t - 2)    # scalar first: gates next step's s_mid
                if live(t - 3):
                    s_mid(t - 3)      # DVE: s_a, then u (waits gpsimd s_b0)
                if live(t - 4):
                    s_act(t - 4)      # scalar: a2 + b2 Prelus
                if live(t - 6):
                    s_dma(t - 6)      # sync queue

    return nc, go


def derive_consts(conv_w, bias, up_filter, down_filter):
    f = np.asarray(up_filter, dtype=np.float64).reshape(-1)
    d = np.asarray(down_filter, dtype=np.float64).reshape(-1)
    fk = (f * 4.0)[::-1]
    fd = d[::-1]
    assert abs(fk[1] - fk[3]) < 1e-6 * max(1.0, abs(fk[1])), "up filter not symmetric"
    assert abs(fk[0] - fk[4]) < 1e-6 * max(1.0, abs(fk[0])), "up filter not symmetric"
    fk0, fk1, fk2 = float(fk[0]), float(fk[1]), float(fk[2])
    assert fk0 != 0.0
    ratio = fk2 / fk0

    # partition index q = 2*ci + g (g = h-half); output partition 2*co + g
    cw = np.asarray(conv_w, dtype=np.float32)  # [co, ci, 1, K]
    wb = np.zeros((K, 128, 128), dtype=np.float16)
    for k in range(K):
        wk = cw[:, :, 0, k].T.astype(np.float16)  # [ci, co]
        wb[k, 0::2, 0::2] = wk
        wb[k, 1::2, 1::2] = wk

    bcol = np.repeat(np.asarray(bias, dtype=np.float32), 2).reshape(128, 1)

    # comb taps are plain fd (fk scales are applied inside the Prelus)
    eye = np.eye(128, dtype=np.float32)
    dg = np.stack(
        [
            np.float32(fd[0]) * eye,
            np.float32(fd[1]) * eye,
            np.float32(fd[2]) * eye,
            np.float32(fd[3]) * eye,
        ]
    ).astype(np.float16)

    return {
        "wb": wb,
        "bcol": bcol,
        "dg": dg,
        "ratio": ratio,
        "fk0": fk0,
        "fk1": fk1,
        "fk2": fk2,
    }


_CACHE = {}


def _get_compiled(consts_key, ratio, fk0, fk1, fk2):
    if consts_key in _CACHE:
        return _CACHE[consts_key]
    nc, go = build_program()
    go(ratio, fk0, fk1, fk2)
    nc.compile()
    _CACHE[consts_key] = nc
    return nc


def run(x, conv_w, bias, up_filter, down_filter, trace=False, **trace_kw):
    x = np.asarray(x, dtype=np.float32)
    c = derive_consts(conv_w, bias, up_filter, down_filter)

    key = (float(c["ratio"]), float(c["fk0"]), float(c["fk1"]), float(c["fk2"]))
    nc = _get_compiled(key, c["ratio"], c["fk0"], c["fk1"], c["fk2"])

    in_maps = []
    for i in range(N_CORES):
        in_maps.append(
            {
                "x": np.ascontiguousarray(x[i]),
                "wb": c["wb"],
                "bcol": c["bcol"],
                "dg": c["dg"],
            }
        )
    res = run_bass_kernel_spmd(
        nc, in_maps, list(range(N_CORES)), trace=trace, **trace_kw
    )
    out = np.stack([res.results[i]["out"] for i in range(N_CORES)], axis=0)
    return out.astype(np.float32), res


def kernel(x, conv_w, bias, up_filter, down_filter):
    out, _ = run(x, conv_w, bias, up_filter, down_filter)
    return out
